# revision 10
# baseline (speedup 1.0000x reference)
"""Distributed attention kernel for one TRN2 chip (8 NeuronCores).

Problem: multi-head cross-attention
  B=4, TQ=512, TKV=4096, D=1024, H=8 heads (head_dim=128)

Sharding (data-parallel x tensor-parallel, per the hint):
  core c in 0..7 -> (batch b = c % 4, head-group g = c // 4)
  Each core computes heads [4g, 4g+4) for its batch: Wq/Wk/Wv column
  shards, Wo row shard, then a pair AllReduce (c <-> c+4 partners) sums
  the two head-group partial outputs.

Device layout (per core; everything transposed so no on-device
transposes are needed - the host passes x^T and mask^T):
  Q^T[dh, t]  = Wq_g^T x_q^T          (4 head-blocks x 8 k-chunks)
  K^T[dh, T]  = Wk_g^T x_kv^T
  V[T, dh]    = x_kv Wv_g             (from x_kv^T chunks as lhsT)
  S^T[T, t]   = K^T_h(block)^T Q^T_h  per head, 32 T-blocks
  P^T         = exp(S^T/sqrt(128)) * mask^T   (no max-subtraction needed:
                scores are O(1) so exp cannot overflow/underflow)
  U^T[dh, t] += V_h(block)^T P^T      accumulated over T-blocks in PSUM
  den[1, t]  += ones^T P^T            (PE ones-matmul = partition sum)
  U^T *= 1/max(den, tiny)             (rows with all-false mask give
                U = 0 exactly, so they stay 0 like the reference wipe)
  out^T[o, t] = Wo_g^T U^T (+ bo on group 0 only), pair AllReduce, DMA out.

Matmul inputs are bf16 (PE 4x faster than fp32); PSUM accumulation,
softmax denominators and reciprocal stay fp32.
"""

import sys

if "/opt/trn_rl_repo" not in sys.path:
    sys.path.insert(0, "/opt/trn_rl_repo")

import numpy as np
import ml_dtypes
from contextlib import ExitStack

B, TQ, TKV, D, H = 4, 512, 4096, 1024, 8
HD = D // H            # 128 head dim
NCORES = 8
GH = H // 2            # heads per core = 4
GD = GH * HD           # 512 cols per head-group
P = 128
KC = D // P            # 8 contraction chunks
NTB = TKV // P         # 32 T-blocks
NTC = TKV // 512       # 8 T-chunks (DMA granularity)
SCALE = float(1.0 / np.sqrt(HD))

_CACHED_NC = None


def _build_nc():
    from concourse import mybir, bacc
    from concourse.tile import TileContext

    bf = mybir.dt.bfloat16
    f32 = mybir.dt.float32
    AF = mybir.ActivationFunctionType
    OP = mybir.AluOpType

    nc = bacc.Bacc("TRN2", target_bir_lowering=False, debug=False,
                   num_devices=NCORES)

    xqT = nc.dram_tensor("xqT", [D, TQ], bf, kind="ExternalInput")
    xkvT = nc.dram_tensor("xkvT", [D, TKV], bf, kind="ExternalInput")
    maskT = nc.dram_tensor("maskT", [TKV, TQ], bf, kind="ExternalInput")
    Wq = nc.dram_tensor("Wq", [D, GD], bf, kind="ExternalInput")
    Wk = nc.dram_tensor("Wk", [D, GD], bf, kind="ExternalInput")
    Wv = nc.dram_tensor("Wv", [D, GD], bf, kind="ExternalInput")
    Wo = nc.dram_tensor("Wo", [GD, D], bf, kind="ExternalInput")
    bq = nc.dram_tensor("bq", [GD], f32, kind="ExternalInput")
    bk = nc.dram_tensor("bk", [GD], f32, kind="ExternalInput")
    bv = nc.dram_tensor("bv", [GD], f32, kind="ExternalInput")
    bo = nc.dram_tensor("bo", [D], f32, kind="ExternalInput")
    out = nc.dram_tensor("out", [GD, TQ], f32, kind="ExternalOutput")

    with TileContext(nc) as tc:
        with ExitStack() as ctx:
            persist = ctx.enter_context(tc.tile_pool(name="persist", bufs=1))
            kvchunk = ctx.enter_context(tc.tile_pool(name="kvchunk", bufs=3))
            work = ctx.enter_context(tc.tile_pool(name="work", bufs=3))
            outp = ctx.enter_context(tc.tile_pool(name="outp", bufs=2))
            ppool = ctx.enter_context(
                tc.tile_pool(name="ppool", bufs=2, space="PSUM"))
            spool = ctx.enter_context(
                tc.tile_pool(name="spool", bufs=2, space="PSUM"))
            upool = ctx.enter_context(
                tc.tile_pool(name="upool", bufs=2, space="PSUM"))
            dpool = ctx.enter_context(
                tc.tile_pool(name="dpool", bufs=2, space="PSUM"))
            dram = ctx.enter_context(
                tc.tile_pool(name="dram", bufs=1, space="DRAM"))

            # ---- constants / weights / biases -------------------------
            # DMA emission order matters for time-to-first-matmul: Wq+xq
            # first so the Q projection starts ~6us in, then Wk/Wv, then
            # the kv chunks; mask/Wo are only needed later.
            wq_sb = persist.tile([P, KC, GD], bf)
            xq_sb = persist.tile([P, KC, TQ], bf)
            wq_ap = Wq.ap().rearrange("(kc p) n -> p kc n", p=P)
            xq_ap = xqT.ap().rearrange("(kc p) t -> p kc t", p=P)
            for kc in range(KC):  # split across DMA queues
                nc.sync.dma_start(wq_sb[:, kc, :], wq_ap[:, kc, :])
                nc.sync.dma_start(xq_sb[:, kc, :], xq_ap[:, kc, :])

            bq_sb = persist.tile([P, GH], f32)
            bk_sb = persist.tile([P, GH], f32)
            nc.sync.dma_start(bq_sb[:], bq.ap().rearrange("(h p) -> p h", p=P))
            nc.sync.dma_start(bk_sb[:], bk.ap().rearrange("(h p) -> p h", p=P))
            bv_row = persist.tile([1, GD], f32)
            nc.sync.dma_start(bv_row[:], bv.ap().unsqueeze(0))
            bv_rep = persist.tile([P, GD], f32)
            nc.gpsimd.partition_broadcast(bv_rep[:], bv_row[:])

            ones_bf = persist.tile([P, 1], bf)
            nc.vector.memset(ones_bf[:], 1.0)

            wk_sb = persist.tile([P, KC, GD], bf)
            wv_sb = persist.tile([P, KC, GD], bf)
            wk_ap = Wk.ap().rearrange("(kc p) n -> p kc n", p=P)
            wv_ap = Wv.ap().rearrange("(kc p) n -> p kc n", p=P)
            for kc in range(KC):
                nc.sync.dma_start(wk_sb[:, kc, :], wk_ap[:, kc, :])
                nc.sync.dma_start(wv_sb[:, kc, :], wv_ap[:, kc, :])

            # ---- Q^T = Wq_g^T x_q^T  (+bq) ----------------------------
            qt_sb = persist.tile([P, GH, TQ], bf)
            for db in range(GH):
                ps = ppool.tile([P, TQ], f32, name="proj_ps")
                for kc in range(KC):
                    nc.tensor.matmul(ps[:], wq_sb[:, kc, db * P:(db + 1) * P],
                                     xq_sb[:, kc, :],
                                     start=(kc == 0), stop=(kc == KC - 1))
                nc.vector.tensor_tensor(
                    qt_sb[:, db, :], ps[:],
                    bq_sb[:, db:db + 1].to_broadcast([P, TQ]), OP.add)

            # ---- K^T and V over T-chunks ------------------------------
            kt_sb = persist.tile([P, GH, TKV], bf)
            v_sb = persist.tile([P, NTB, GD], bf)
            mask_sb = persist.tile([P, NTB, TQ], bf)
            bo_sb = persist.tile([P, D // P], f32)
            wo_sb = persist.tile([P, GH, D], bf)
            for tcknk in range(NTC):
                xkv_t = kvchunk.tile([P, KC, 512], bf, tag="xkv")
                nc.sync.dma_start(
                    xkv_t[:],
                    xkvT.ap().rearrange("(kc p) T -> p kc T", p=P)
                    [:, :, tcknk * 512:(tcknk + 1) * 512])
                if tcknk == 1:
                    # queue the bulk "later-phase" loads behind chunks 0-1,
                    # split into pieces so they spread across DMA queues
                    mask_ap = maskT.ap().rearrange("(j p) t -> p j t", p=P)
                    for mg in range(8):
                        nc.sync.dma_start(mask_sb[:, mg * 4:(mg + 1) * 4, :],
                                          mask_ap[:, mg * 4:(mg + 1) * 4, :])
                    wo_ap = Wo.ap().rearrange("(c p) o -> p c o", p=P)
                    for c in range(GH):
                        nc.sync.dma_start(wo_sb[:, c, :], wo_ap[:, c, :])
                    nc.sync.dma_start(
                        bo_sb[:], bo.ap().rearrange("(ob p) -> p ob", p=P))
                for db in range(GH):
                    ps = ppool.tile([P, 512], f32, name="proj_ps")
                    for kc in range(KC):
                        nc.tensor.matmul(ps[:], wk_sb[:, kc, db * P:(db + 1) * P],
                                         xkv_t[:, kc, :],
                                         start=(kc == 0), stop=(kc == KC - 1))
                    nc.vector.tensor_tensor(
                        kt_sb[:, db, tcknk * 512:(tcknk + 1) * 512], ps[:],
                        bk_sb[:, db:db + 1].to_broadcast([P, 512]), OP.add)
                for tb in range(4):
                    ps = ppool.tile([P, 512], f32, name="proj_ps")
                    for kc in range(KC):
                        nc.tensor.matmul(ps[:],
                                         xkv_t[:, kc, tb * P:(tb + 1) * P],
                                         wv_sb[:, kc, :],
                                         start=(kc == 0), stop=(kc == KC - 1))
                    nc.vector.tensor_tensor(
                        v_sb[:, tcknk * 4 + tb, :], ps[:], bv_rep[:], OP.add)

            # ---- attention, flattened (head, T-block) loop ------------
            # S-matmul prefetch crosses head boundaries so the PE never
            # waits for the exp/mask pipeline to drain between heads.
            ut_sb = persist.tile([P, GH, TQ], bf)
            NSTEP = GH * NTB
            s_tiles = {}
            u_tiles = [None] * GH
            den_tiles = [None] * GH

            def s_mm(step):
                h, j = divmod(step, NTB)
                s_ps = spool.tile([P, TQ], f32, name="s_ps", tag="s_ps")
                nc.tensor.matmul(s_ps[:], kt_sb[:, h, j * P:(j + 1) * P],
                                 qt_sb[:, h, :], start=True, stop=True)
                return s_ps

            s_tiles[0] = s_mm(0)
            s_tiles[1] = s_mm(1)
            for step in range(NSTEP):
                h, j = divmod(step, NTB)
                if j == 0:
                    u_tiles[h] = upool.tile([P, TQ], f32, name="u_ps",
                                            tag="u_ps")
                    den_tiles[h] = dpool.tile([1, TQ], f32, name="den_ps",
                                              tag="den_ps")
                s_ps = s_tiles.pop(step)
                praw = work.tile([P, TQ], bf, tag="praw")
                nc.scalar.activation(praw[:], s_ps[:], AF.Exp, scale=SCALE)
                p_t = work.tile([P, TQ], bf, tag="p_t")
                nc.vector.tensor_tensor(p_t[:], praw[:], mask_sb[:, j, :],
                                        OP.mult)
                if step + 2 < NSTEP:
                    s_tiles[step + 2] = s_mm(step + 2)
                nc.tensor.matmul(u_tiles[h][:], v_sb[:, j, h * P:(h + 1) * P],
                                 p_t[:], start=(j == 0), stop=(j == NTB - 1))
                nc.tensor.matmul(den_tiles[h][:], ones_bf[:], p_t[:],
                                 start=(j == 0), stop=(j == NTB - 1))
                if j == NTB - 1:
                    den_sf = work.tile([1, TQ], f32, tag="den_sf")
                    nc.vector.tensor_scalar(den_sf[:], den_tiles[h][:], 1e-30,
                                            None, OP.max)
                    recip = work.tile([1, TQ], f32, tag="recip")
                    nc.vector.reciprocal(recip[:], den_sf[:])
                    recip_rep = work.tile([P, TQ], f32, tag="recip_rep")
                    nc.gpsimd.partition_broadcast(recip_rep[:], recip[:])
                    nc.vector.tensor_tensor(ut_sb[:, h, :], u_tiles[h][:],
                                            recip_rep[:], OP.mult)

            # ---- out^T = Wo_g^T U^T (+bo), pair ReduceScatter ---------
            # RS moves half the bytes of an AllReduce; rank0 of each pair
            # keeps o-rows [0,512), rank1 keeps [512,1024); host concats.
            cc_in = dram.tile([D, TQ], f32)
            cc_out = dram.tile([GD, TQ], f32)
            for ob in range(D // P):
                ps = ppool.tile([P, TQ], f32, name="proj_ps")
                for hc in range(GH):
                    nc.tensor.matmul(ps[:], wo_sb[:, hc, ob * P:(ob + 1) * P],
                                     ut_sb[:, hc, :],
                                     start=(hc == 0), stop=(hc == GH - 1))
                o_sb = outp.tile([P, TQ], f32, tag="o_sb")
                nc.vector.tensor_tensor(
                    o_sb[:], ps[:],
                    bo_sb[:, ob:ob + 1].to_broadcast([P, TQ]), OP.add)
                nc.sync.dma_start(cc_in[ob * P:(ob + 1) * P, :], o_sb[:])

            nc.gpsimd.collective_compute(
                "ReduceScatter", mybir.AluOpType.add,
                replica_groups=[[0, 4], [1, 5], [2, 6], [3, 7]],
                ins=[cc_in.opt()], outs=[cc_out.opt()],
            )
            nc.sync.dma_start(out.ap(), cc_out[:])

    nc.finalize()
    return nc


def _shard_inputs(inputs_q, inputs_kv, attention_mask, Wq, bq, Wk, bk, Wv, bv,
                  Wo, bo):
    bf16 = ml_dtypes.bfloat16
    f32 = np.float32
    in_maps = []
    xqT = [np.ascontiguousarray(inputs_q[b].T).astype(bf16) for b in range(B)]
    xkvT = [np.ascontiguousarray(inputs_kv[b].T).astype(bf16) for b in range(B)]
    maskT = [np.ascontiguousarray(attention_mask[b].T).astype(bf16)
             for b in range(B)]
    for c in range(NCORES):
        b, g = c % B, c // B
        sl = slice(g * GD, (g + 1) * GD)
        in_maps.append({
            "xqT": xqT[b],
            "xkvT": xkvT[b],
            "maskT": maskT[b],
            "Wq": np.ascontiguousarray(Wq[:, sl]).astype(bf16),
            "Wk": np.ascontiguousarray(Wk[:, sl]).astype(bf16),
            "Wv": np.ascontiguousarray(Wv[:, sl]).astype(bf16),
            "Wo": np.ascontiguousarray(Wo[sl, :]).astype(bf16),
            "bq": np.ascontiguousarray(bq[sl]).astype(f32),
            "bk": np.ascontiguousarray(bk[sl]).astype(f32),
            "bv": np.ascontiguousarray(bv[sl]).astype(f32),
            "bo": (bo.astype(f32) if g == 0 else np.zeros(D, f32)),
        })
    return in_maps


def kernel(_trace=False, **inputs):
    global _CACHED_NC
    from concourse import bass_utils

    arrs = {k: np.asarray(v) for k, v in inputs.items()}
    in_maps = _shard_inputs(**arrs)

    if _CACHED_NC is None:
        _CACHED_NC = _build_nc()

    res = bass_utils.run_bass_kernel_spmd(
        _CACHED_NC, in_maps, core_ids=list(range(NCORES)), trace=_trace)

    full = np.empty((B, TQ, D), np.float32)
    for b in range(B):
        # pair (b, b+4) ReduceScatter: core b holds o-rows [0,512),
        # core b+4 holds [512,1024) of the summed transposed output
        full[b, :, :GD] = res.results[b]["out"].T
        full[b, :, GD:] = res.results[b + 4]["out"].T
    if _trace:
        return full, res
    return full
